# revision 10
# baseline (speedup 1.0000x reference)
"""Trainium2 Bass kernel for nn_ExRestSelfAtten (sparse window attention).

Math reduction (exact):
  reference softmax is over a singleton axis -> attn_w == ones exactly,
  so Wq/Wk are dead and
    out[b,t,:] = (sum_{|j-t|<=5} relu(x[b,j]@W1+b1) + winsum(pos)[t]) @ (Wv@Wo) + bo
  i.e.  out[b,t,o] = sum_j sum_h M[t,j] * relu(x@W1+b1)[b,j,h] * Wvo[h,o] + c[t,o]

Device pipeline per core (B_core=4096 batches -> 45056 tokens):
  DMA x (tok,128) tiles -> PE transpose -> SBUF -> MM1 (lhsT=W1) -> relu+b1
  -> MM2: 11 PSUM-accumulated matmuls, lhsT=wfold_j (32h x 22(t,o)), rhs = R[:, s==j]
  -> +c bias -> PE transpose to batch-major -> DMA out.
"""

import os
import sys
import numpy as np
from contextlib import ExitStack

sys.path.insert(0, "/opt/trn_rl_repo")

B, S, D, H, O = 32768, 11, 128, 32, 2
A = 5
WIN = 2 * A + 1
NCORES = 8
NB = B // NCORES              # 4096 batches per core
NTOK = NB * S                 # 45056 tokens per core
TO = S * O                    # 22 = flattened (t, o)

OCTS = 8                      # pipeline stages per core
BOCT = NB // OCTS             # 512 batches per octant
TOKOCT = BOCT * S             # 5632 tokens per octant
TILES_OCT = TOKOCT // 128     # 44 tiles of 128 tokens
CH = 512                      # tokens per chunk (matmul N)
NCH = TOKOCT // CH            # 11 chunks per octant

_CACHE = {}


def _build_nc():
    import concourse.bass as bass
    import concourse.tile as tile
    from concourse import bacc, mybir

    f32 = mybir.dt.float32
    Relu = mybir.ActivationFunctionType.Relu
    Ident = mybir.ActivationFunctionType.Identity

    nc = bacc.Bacc()
    x_ext = nc.dram_tensor("x", [NTOK, D], f32, kind="ExternalInput")
    w1_ext = nc.dram_tensor("w1", [D, H], f32, kind="ExternalInput")
    wfold_ext = nc.dram_tensor("wfold", [H, S * TO], f32, kind="ExternalInput")
    b1_ext = nc.dram_tensor("b1v", [H, 1], f32, kind="ExternalInput")
    cvec_ext = nc.dram_tensor("cvec", [TO, 1], f32, kind="ExternalInput")
    ident_ext = nc.dram_tensor("ident", [128, 128], f32, kind="ExternalInput")
    out_ext = nc.dram_tensor("out", [NB, TO], f32, kind="ExternalOutput")

    with tile.TileContext(nc) as tc, ExitStack() as ctx:
        consts = ctx.enter_context(tc.tile_pool(name="consts", bufs=1))
        xpool = ctx.enter_context(tc.tile_pool(name="xpool", bufs=2))
        rpool = ctx.enter_context(tc.tile_pool(name="rpool", bufs=2))
        xtpool = ctx.enter_context(tc.tile_pool(name="xtpool", bufs=3))
        osbpool = ctx.enter_context(tc.tile_pool(name="osbpool", bufs=2))
        otpool = ctx.enter_context(tc.tile_pool(name="otpool", bufs=2))
        ps_xt = ctx.enter_context(tc.tile_pool(name="ps_xt", bufs=2, space="PSUM"))
        ps_y = ctx.enter_context(tc.tile_pool(name="ps_y", bufs=2, space="PSUM"))
        ps_o2 = ctx.enter_context(tc.tile_pool(name="ps_o2", bufs=1, space="PSUM"))
        ps_ot = ctx.enter_context(tc.tile_pool(name="ps_ot", bufs=1, space="PSUM"))

        w1_sb = consts.tile([D, H], f32)
        nc.sync.dma_start(out=w1_sb, in_=w1_ext[:])
        wfold_sb = consts.tile([H, S * TO], f32)
        nc.sync.dma_start(out=wfold_sb, in_=wfold_ext[:])
        b1_sb = consts.tile([H, 1], f32)
        nc.sync.dma_start(out=b1_sb, in_=b1_ext[:])
        cvec_sb = consts.tile([TO, 1], f32)
        nc.sync.dma_start(out=cvec_sb, in_=cvec_ext[:])
        ident_sb = consts.tile([128, 128], f32)
        nc.sync.dma_start(out=ident_sb, in_=ident_ext[:])

        # Token layout within an octant: partition-major.  Partition p holds
        # tokens [p*44, (p+1)*44) of the octant; since 44 = 4*11, partition p
        # holds exactly batches [4p, 4p+4).  Token (p, i) with i = bl*11 + s.
        for oct_i in range(OCTS):
            # ---- load 5632 tokens (512 batches); contiguous per partition ----
            xq = xpool.tile([128, TILES_OCT, D], f32)
            src = x_ext[oct_i * TOKOCT:(oct_i + 1) * TOKOCT, :].rearrange(
                "(p i) f -> p (i f)", p=128)
            nc.sync.dma_start(out=xq.rearrange("p i f -> p (i f)"), in_=src)

            Rq = rpool.tile([H, TOKOCT], f32)
            for c in range(NCH):
                # transpose 4 x (128 tok,128 f) -> (128 f, 512 tok)
                # chunk c covers i in [4c, 4c+4); R free index = 128*i + p
                xtP = ps_xt.tile([128, CH], f32)
                for j in range(4):
                    nc.tensor.transpose(
                        xtP[:, j * 128:(j + 1) * 128], xq[:, c * 4 + j, :], ident_sb)
                xt = xtpool.tile([128, CH], f32)
                nc.scalar.copy(xt, xtP)
                # y = W1.T @ xt  -> (32 h, 512 tok)
                yP = ps_y.tile([H, CH], f32)
                nc.tensor.matmul(yP, w1_sb, xt, start=True, stop=True)
                # R = relu(y + b1)
                nc.scalar.activation(
                    out=Rq[:, c * CH:(c + 1) * CH], in_=yP, func=Relu,
                    bias=b1_sb, scale=1.0)

            # ---- MM2: out2[(t,o), (bl,p)] = sum_s wfold_s.T @ R[:, s slice] ----
            # R free index 128*i + p with i = bl*11 + s -> expose (bl, s, p)
            o2 = ps_o2.tile([TO, BOCT], f32)
            Rview = Rq.rearrange("h (bl s q) -> h bl s q", bl=4, s=S)
            for bl in range(4):
                for j in range(S):
                    nc.tensor.matmul(
                        o2[:, bl * 128:(bl + 1) * 128],
                        wfold_sb[:, j * TO:(j + 1) * TO], Rview[:, bl, j],
                        start=(j == 0), stop=(j == S - 1))
            osb = osbpool.tile([TO, BOCT], f32)
            nc.scalar.activation(out=osb, in_=o2, func=Ident, bias=cvec_sb, scale=1.0)

            # ---- transpose to batch-major: column (bl,p) is batch 4p+bl ----
            oTp = ps_ot.tile([128, 4, TO], f32)
            for blk in range(4):
                nc.tensor.transpose(
                    oTp[:, blk, :], osb[:, blk * 128:(blk + 1) * 128],
                    ident_sb[:TO, :TO])
            outT = otpool.tile([128, 4, TO], f32)
            nc.scalar.copy(outT, oTp)
            # outT[p, blk, :] is batch 4p + blk -> rows (p blk) in order
            dst = out_ext[oct_i * BOCT:(oct_i + 1) * BOCT, :].rearrange(
                "(p blk) to -> p (blk to)", p=128)
            nc.sync.dma_start(out=dst, in_=outT.rearrange("p blk to -> p (blk to)"))

    nc.finalize()
    return nc


def _get_nc():
    if "nc" not in _CACHE:
        _CACHE["nc"] = _build_nc()
    return _CACHE["nc"]


def _fold_weights(W1, b1, Wv, pos_enc, Wo, bo):
    Wvo = Wv.astype(np.float64) @ Wo.astype(np.float64)          # (32, 2)
    t_idx = np.arange(S)
    M = (np.abs(t_idx[:, None] - t_idx[None, :]) <= A).astype(np.float64)  # (t, j)
    # wfold[h, j*22 + t*2+o] = M[t,j] * Wvo[h,o]
    wfold = np.einsum("tj,ho->hjto", M, Wvo).reshape(H, S * TO)
    pos = pos_enc.reshape(S, H).astype(np.float64)
    cvec = (M @ pos) @ Wvo + bo.reshape(1, O).astype(np.float64)  # (t, o)
    return (wfold.astype(np.float32),
            cvec.reshape(TO, 1).astype(np.float32),
            b1.reshape(H, 1).astype(np.float32))


def kernel(x, W1, b1, Wq, Wk, Wv, pos_enc, Wo, bo):
    from concourse.bass_utils import run_bass_kernel_spmd

    x = np.ascontiguousarray(np.asarray(x, dtype=np.float32))
    assert x.shape == (B, S, D), x.shape
    wfold, cvec, b1v = _fold_weights(
        np.asarray(W1, np.float32), np.asarray(b1, np.float32),
        np.asarray(Wv, np.float32), np.asarray(pos_enc, np.float32),
        np.asarray(Wo, np.float32), np.asarray(bo, np.float32))
    ident = np.eye(128, dtype=np.float32)
    w1np = np.ascontiguousarray(np.asarray(W1, np.float32))

    nc = _get_nc()
    in_maps = []
    for i in range(NCORES):
        shard = x[i * NB:(i + 1) * NB].reshape(NTOK, D)
        in_maps.append({
            "x": shard, "w1": w1np, "wfold": wfold, "b1v": b1v,
            "cvec": cvec, "ident": ident,
        })

    res = run_bass_kernel_spmd(nc, in_maps, list(range(NCORES)), trace=False)
    _CACHE["last_result"] = res

    out = np.concatenate([r["out"] for r in res.results], axis=0)
    out = out.reshape(B, S, O)
    attn_w = np.ones((B, S, 1, WIN), dtype=np.float32)
    return out, attn_w


# revision 12
# speedup vs baseline: 1.5696x; 1.5696x over previous
"""Trainium2 Bass kernel for nn_ExRestSelfAtten (sparse window attention).

Math reduction (exact):
  reference softmax is over a singleton axis -> attn_w == ones exactly,
  so Wq/Wk are dead and
    out[b,t,o] = sum_s sum_h M[t,s] * relu(x@W1+b1)[b,s,h] * Wvo[h,o] + c[t,o]
  with M[t,s] = 1{|t-s|<=5}, Wvo = Wv@Wo, c folding pos_enc/bo.

Device pipeline per core (4096 batches = 45056 tokens, 8 octants of 512):
  - x is split on host into fp16 hi/lo planes (x = hi + lo + O(2^-22));
    both planes are DMA-xbar-transposed into SBUF feature-major.
  - MM1 (fp16, 3 terms: hi*W1hi + hi*W1lo + lo*W1hi) contracts f=128 per
    sequence position s, writing PSUM partition strips so partitions become
    (s%4)*32+h; relu+b1 -> R2[g] (g = s//4).
  - MM2 (f32) contracts (s,h) in 3 K-chunks with W2g[(s,h),(t,o)] =
    M[t,s]*Wvo[h,o], accumulating out[(t,o), batch] in PSUM.
  - +c bias, PE-transpose to batch-major, DMA out.
"""

import os
import sys
import numpy as np
from contextlib import ExitStack

sys.path.insert(0, "/opt/trn_rl_repo")

B, S, D, H, O = 32768, 11, 128, 32, 2
A = 5
WIN = 2 * A + 1
NCORES = 8
NB = B // NCORES              # 4096 batches per core
NTOK = NB * S                 # 45056 tokens per core
TO = S * O                    # 22 = flattened (t, o)

OCTS = 8                      # pipeline stages per core
BOCT = NB // OCTS             # 512 batches per octant
TOKOCT = BOCT * S             # 5632 tokens per octant
KCH = [(0, 4), (4, 8), (8, 11)]   # s-ranges of the 3 (s,h) K-chunks

_CACHE = {}


def _build_nc():
    import concourse.bass as bass
    import concourse.tile as tile
    from concourse import bacc, mybir

    f32 = mybir.dt.float32
    f16 = mybir.dt.float16
    Relu = mybir.ActivationFunctionType.Relu
    Ident = mybir.ActivationFunctionType.Identity

    nc = bacc.Bacc()
    xhi_ext = nc.dram_tensor("xhi", [NTOK, D], f16, kind="ExternalInput")
    xlo_ext = nc.dram_tensor("xlo", [NTOK, D], f16, kind="ExternalInput")
    w1hi_ext = nc.dram_tensor("w1hi", [D, H], f16, kind="ExternalInput")
    w1lo_ext = nc.dram_tensor("w1lo", [D, H], f16, kind="ExternalInput")
    w2_ext = nc.dram_tensor("w2", [128, 3 * TO], f32, kind="ExternalInput")
    b1_ext = nc.dram_tensor("b1r", [128, 1], f32, kind="ExternalInput")
    cvec_ext = nc.dram_tensor("cvec", [TO, 1], f32, kind="ExternalInput")
    id22_ext = nc.dram_tensor("id22", [TO, TO], f32, kind="ExternalInput")
    out_ext = nc.dram_tensor("out", [NB, TO], f32, kind="ExternalOutput")

    with tile.TileContext(nc) as tc, ExitStack() as ctx:
        consts = ctx.enter_context(tc.tile_pool(name="consts", bufs=1))
        xtpool = ctx.enter_context(tc.tile_pool(name="xtpool", bufs=2))
        r2pool = ctx.enter_context(tc.tile_pool(name="r2pool", bufs=2))
        osbpool = ctx.enter_context(tc.tile_pool(name="osbpool", bufs=2))
        otpool = ctx.enter_context(tc.tile_pool(name="otpool", bufs=2))
        ps_r2 = ctx.enter_context(tc.tile_pool(name="ps_r2", bufs=1, space="PSUM"))
        ps_o2 = ctx.enter_context(tc.tile_pool(name="ps_o2", bufs=2, space="PSUM"))
        ps_ot = ctx.enter_context(tc.tile_pool(name="ps_ot", bufs=2, space="PSUM"))

        w1hi_sb = consts.tile([D, H], f16)
        nc.sync.dma_start(out=w1hi_sb, in_=w1hi_ext[:])
        w1lo_sb = consts.tile([D, H], f16)
        nc.sync.dma_start(out=w1lo_sb, in_=w1lo_ext[:])
        w2_sb = consts.tile([128, 3 * TO], f32)
        nc.sync.dma_start(out=w2_sb, in_=w2_ext[:])
        b1_sb = consts.tile([128, 1], f32)
        nc.sync.dma_start(out=b1_sb, in_=b1_ext[:])
        cvec_sb = consts.tile([TO, 1], f32)
        nc.sync.dma_start(out=cvec_sb, in_=cvec_ext[:])
        id22_sb = consts.tile([TO, TO], f32)
        nc.sync.dma_start(out=id22_sb, in_=id22_ext[:])

        for oct_i in range(OCTS):
            tok0 = oct_i * TOKOCT
            # ---- xbar-transposed loads: (5632 tok, 128 f) -> (128 f, 5632) ----
            xth = xtpool.tile([D, TOKOCT], f16)
            nc.sync.dma_start_transpose(
                out=xth, in_=xhi_ext[tok0:tok0 + TOKOCT, :])
            xtl = xtpool.tile([D, TOKOCT], f16)
            nc.sync.dma_start_transpose(
                out=xtl, in_=xlo_ext[tok0:tok0 + TOKOCT, :])

            # token t = 44p + 11bl + s ; output column n = bl*128 + p = batch
            # (4p+bl).  View: (f, p, bl, s)
            xthv = xth.rearrange("f (p bl s) -> f s bl p", p=128, bl=4, s=S)
            xtlv = xtl.rearrange("f (p bl s) -> f s bl p", p=128, bl=4, s=S)

            # ---- MM1 into (s%4)*32+h partition strips, one PSUM tile per g --
            r2ps = [ps_r2.tile([128, BOCT], f32, name=f"r2ps{g}_{oct_i}",
                               tag=f"r2ps{g}")
                    for g in range(3)]
            for s in range(S):
                g, sm = s // 4, s % 4
                outap = r2ps[g][32 * sm:32 * sm + 32, :]
                tp = (0, 32 * sm)
                nc.tensor.matmul(outap, w1hi_sb, xthv[:, s], start=True,
                                 stop=False, tile_position=tp)
                nc.tensor.matmul(outap, w1lo_sb, xthv[:, s], start=False,
                                 stop=False, tile_position=tp)
                nc.tensor.matmul(outap, w1hi_sb, xtlv[:, s], start=False,
                                 stop=True, tile_position=tp)

            # ---- relu + b1 -> SBUF ----
            r2 = [r2pool.tile([128, BOCT], f32, name=f"r2{g}_{oct_i}",
                              tag=f"r2{g}")
                  for g in range(3)]
            for g, (s0, s1) in enumerate(KCH):
                np_ = 32 * (s1 - s0)
                nc.scalar.activation(
                    out=r2[g][:np_, :], in_=r2ps[g][:np_, :], func=Relu,
                    bias=b1_sb[:np_], scale=1.0)

            # ---- MM2: 3 K-chunks over (s,h) ----
            o2 = ps_o2.tile([TO, BOCT], f32)
            for g, (s0, s1) in enumerate(KCH):
                np_ = 32 * (s1 - s0)
                nc.tensor.matmul(
                    o2, w2_sb[:np_, g * TO:(g + 1) * TO], r2[g][:np_, :],
                    start=(g == 0), stop=(g == 2))
            osb = osbpool.tile([TO, BOCT], f32)
            nc.scalar.activation(out=osb, in_=o2, func=Ident, bias=cvec_sb,
                                 scale=1.0)

            # ---- transpose to batch-major: column n=bl*128+p is batch 4p+bl -
            oTp = ps_ot.tile([128, 4, TO], f32)
            for blk in range(4):
                nc.tensor.transpose(
                    oTp[:, blk, :], osb[:, blk * 128:(blk + 1) * 128], id22_sb)
            outT = otpool.tile([128, 4, TO], f32)
            nc.scalar.copy(outT, oTp)
            # outT[p, blk, :] is batch 4p + blk -> rows (p blk) in order
            dst = out_ext[oct_i * BOCT:(oct_i + 1) * BOCT, :].rearrange(
                "(p blk) to -> p (blk to)", p=128)
            nc.sync.dma_start(out=dst, in_=outT.rearrange("p blk to -> p (blk to)"))

    nc.finalize()
    return nc


def _get_nc():
    if "nc" not in _CACHE:
        _CACHE["nc"] = _build_nc()
    return _CACHE["nc"]


def _fold_weights(W1, b1, Wv, pos_enc, Wo, bo):
    Wvo = Wv.astype(np.float64) @ Wo.astype(np.float64)          # (32, 2)
    t_idx = np.arange(S)
    M = (np.abs(t_idx[:, None] - t_idx[None, :]) <= A).astype(np.float64)  # (t, s)
    # W2[(s-s0)*32+h, g*22 + t*2+o] = M[t, s] * Wvo[h, o]
    w2 = np.zeros((128, 3 * TO), np.float64)
    for g, (s0, s1) in enumerate(KCH):
        blk = np.einsum("st,ho->shto", M.T[s0:s1], Wvo).reshape(
            (s1 - s0) * H, TO)
        w2[:(s1 - s0) * H, g * TO:(g + 1) * TO] = blk
    pos = pos_enc.reshape(S, H).astype(np.float64)
    cvec = (M @ pos) @ Wvo + bo.reshape(1, O).astype(np.float64)  # (t, o)
    b1r = np.tile(b1.reshape(1, H), (4, 1)).reshape(128, 1)
    w1hi = W1.astype(np.float16)
    w1lo = (W1.astype(np.float64) - w1hi.astype(np.float64)).astype(np.float16)
    return (w2.astype(np.float32), cvec.reshape(TO, 1).astype(np.float32),
            b1r.astype(np.float32), w1hi, w1lo)


def kernel(x, W1, b1, Wq, Wk, Wv, pos_enc, Wo, bo):
    from concourse.bass_utils import run_bass_kernel_spmd

    x = np.asarray(x, dtype=np.float32)
    assert x.shape == (B, S, D), x.shape
    w2, cvec, b1r, w1hi, w1lo = _fold_weights(
        np.asarray(W1, np.float32), np.asarray(b1, np.float32),
        np.asarray(Wv, np.float32), np.asarray(pos_enc, np.float32),
        np.asarray(Wo, np.float32), np.asarray(bo, np.float32))
    id22 = np.eye(TO, dtype=np.float32)

    xf = x.reshape(B * S, D)
    xhi = xf.astype(np.float16)
    xlo = (xf - xhi.astype(np.float32)).astype(np.float16)

    nc = _get_nc()
    in_maps = []
    for i in range(NCORES):
        sl = slice(i * NTOK, (i + 1) * NTOK)
        in_maps.append({
            "xhi": np.ascontiguousarray(xhi[sl]),
            "xlo": np.ascontiguousarray(xlo[sl]),
            "w1hi": w1hi, "w1lo": w1lo, "w2": w2, "b1r": b1r,
            "cvec": cvec, "id22": id22,
        })

    res = run_bass_kernel_spmd(nc, in_maps, list(range(NCORES)), trace=False)
    _CACHE["last_result"] = res

    out = np.concatenate([r["out"] for r in res.results], axis=0)
    out = out.reshape(B, S, O)
    attn_w = np.ones((B, S, 1, WIN), dtype=np.float32)
    return out, attn_w


# revision 13
# speedup vs baseline: 1.5891x; 1.0125x over previous
"""Trainium2 Bass kernel for nn_ExRestSelfAtten (sparse window attention).

Math reduction (exact):
  reference softmax is over a singleton axis -> attn_w == ones exactly,
  so Wq/Wk are dead and
    out[b,t,o] = sum_s sum_h M[t,s] * relu(x@W1+b1)[b,s,h] * Wvo[h,o] + c[t,o]
  with M[t,s] = 1{|t-s|<=5}, Wvo = Wv@Wo, c folding pos_enc/bo.

Device pipeline per core (4096 batches = 45056 tokens, 8 octants of 512):
  - x is split on host into fp16 hi/lo planes (x = hi + lo + O(2^-22));
    both planes are DMA-xbar-transposed into SBUF feature-major.
  - MM1 (fp16, 3 terms: hi*W1hi + hi*W1lo + lo*W1hi) contracts f=128 per
    sequence position s, writing PSUM partition strips so partitions become
    (s%4)*32+h; relu+b1 -> R2[g] (g = s//4).
  - MM2 (f32) contracts (s,h) in 3 K-chunks with W2g[(s,h),(t,o)] =
    M[t,s]*Wvo[h,o], accumulating out[(t,o), batch] in PSUM.
  - +c bias, PE-transpose to batch-major, DMA out.
"""

import os
import sys
import numpy as np
from contextlib import ExitStack

sys.path.insert(0, "/opt/trn_rl_repo")

B, S, D, H, O = 32768, 11, 128, 32, 2
A = 5
WIN = 2 * A + 1
NCORES = 8
NB = B // NCORES              # 4096 batches per core
NTOK = NB * S                 # 45056 tokens per core
TO = S * O                    # 22 = flattened (t, o)

OCTS = 8                      # pipeline stages per core
BOCT = NB // OCTS             # 512 batches per octant
TOKOCT = BOCT * S             # 5632 tokens per octant
KCH = [(0, 4), (4, 8), (8, 11)]   # s-ranges of the 3 (s,h) K-chunks

_CACHE = {}


def _build_nc():
    import concourse.bass as bass
    import concourse.tile as tile
    from concourse import bacc, mybir

    f32 = mybir.dt.float32
    f16 = mybir.dt.float16
    Relu = mybir.ActivationFunctionType.Relu
    Ident = mybir.ActivationFunctionType.Identity

    nc = bacc.Bacc()
    xhi_ext = nc.dram_tensor("xhi", [NTOK, D], f16, kind="ExternalInput")
    xlo_ext = nc.dram_tensor("xlo", [NTOK, D], f16, kind="ExternalInput")
    w1hi_ext = nc.dram_tensor("w1hi", [D, H], f16, kind="ExternalInput")
    w1lo_ext = nc.dram_tensor("w1lo", [D, H], f16, kind="ExternalInput")
    w2_ext = nc.dram_tensor("w2", [128, 3 * TO], f32, kind="ExternalInput")
    b1_ext = nc.dram_tensor("b1r", [128, 1], f32, kind="ExternalInput")
    cvec_ext = nc.dram_tensor("cvec", [TO, 1], f32, kind="ExternalInput")
    id22_ext = nc.dram_tensor("id22", [TO, TO], f32, kind="ExternalInput")
    out_ext = nc.dram_tensor("out", [NB, TO], f32, kind="ExternalOutput")

    with tile.TileContext(nc) as tc, ExitStack() as ctx:
        consts = ctx.enter_context(tc.tile_pool(name="consts", bufs=1))
        xtpool = ctx.enter_context(tc.tile_pool(name="xtpool", bufs=2))
        r2pool = ctx.enter_context(tc.tile_pool(name="r2pool", bufs=2))
        osbpool = ctx.enter_context(tc.tile_pool(name="osbpool", bufs=2))
        otpool = ctx.enter_context(tc.tile_pool(name="otpool", bufs=2))
        ps_r2 = ctx.enter_context(tc.tile_pool(name="ps_r2", bufs=2, space="PSUM"))
        ps_o2 = ctx.enter_context(tc.tile_pool(name="ps_o2", bufs=1, space="PSUM"))
        ps_ot = ctx.enter_context(tc.tile_pool(name="ps_ot", bufs=1, space="PSUM"))

        w1hi_sb = consts.tile([D, H], f16)
        nc.sync.dma_start(out=w1hi_sb, in_=w1hi_ext[:])
        w1lo_sb = consts.tile([D, H], f16)
        nc.sync.dma_start(out=w1lo_sb, in_=w1lo_ext[:])
        w2_sb = consts.tile([128, 3 * TO], f32)
        nc.sync.dma_start(out=w2_sb, in_=w2_ext[:])
        b1_sb = consts.tile([128, 1], f32)
        nc.sync.dma_start(out=b1_sb, in_=b1_ext[:])
        cvec_sb = consts.tile([TO, 1], f32)
        nc.sync.dma_start(out=cvec_sb, in_=cvec_ext[:])
        id22_sb = consts.tile([TO, TO], f32)
        nc.sync.dma_start(out=id22_sb, in_=id22_ext[:])

        for oct_i in range(OCTS):
            tok0 = oct_i * TOKOCT
            # ---- xbar-transposed loads: (5632 tok, 128 f) -> (128 f, 5632) ----
            xth = xtpool.tile([D, TOKOCT], f16)
            nc.sync.dma_start_transpose(
                out=xth, in_=xhi_ext[tok0:tok0 + TOKOCT, :])
            xtl = xtpool.tile([D, TOKOCT], f16)
            nc.sync.dma_start_transpose(
                out=xtl, in_=xlo_ext[tok0:tok0 + TOKOCT, :])

            # token t = 44p + 11bl + s ; output column n = bl*128 + p = batch
            # (4p+bl).  View: (f, p, bl, s)
            xthv = xth.rearrange("f (p bl s) -> f s bl p", p=128, bl=4, s=S)
            xtlv = xtl.rearrange("f (p bl s) -> f s bl p", p=128, bl=4, s=S)

            # ---- MM1 into (s%4)*32+h partition strips, one PSUM tile per g --
            r2ps = [ps_r2.tile([128, BOCT], f32, name=f"r2ps{g}_{oct_i}",
                               tag=f"r2ps{g}")
                    for g in range(3)]
            # group by stationary operand to minimize weight reloads
            for phase, (wsb, xv, st, sp) in enumerate([
                    (w1hi_sb, xthv, True, False),
                    (w1hi_sb, xtlv, False, False),
                    (w1lo_sb, xthv, False, True)]):
                for s in range(S):
                    g, sm = s // 4, s % 4
                    nc.tensor.matmul(
                        r2ps[g][32 * sm:32 * sm + 32, :], wsb, xv[:, s],
                        start=st, stop=sp, tile_position=(0, 32 * sm))

            # ---- relu + b1 -> SBUF ----
            r2 = [r2pool.tile([128, BOCT], f32, name=f"r2{g}_{oct_i}",
                              tag=f"r2{g}")
                  for g in range(3)]
            for g, (s0, s1) in enumerate(KCH):
                np_ = 32 * (s1 - s0)
                nc.scalar.activation(
                    out=r2[g][:np_, :], in_=r2ps[g][:np_, :], func=Relu,
                    bias=b1_sb[:np_], scale=1.0)

            # ---- MM2: 3 K-chunks over (s,h) ----
            o2 = ps_o2.tile([TO, BOCT], f32)
            for g, (s0, s1) in enumerate(KCH):
                np_ = 32 * (s1 - s0)
                nc.tensor.matmul(
                    o2, w2_sb[:np_, g * TO:(g + 1) * TO], r2[g][:np_, :],
                    start=(g == 0), stop=(g == 2))
            osb = osbpool.tile([TO, BOCT], f32)
            nc.scalar.activation(out=osb, in_=o2, func=Ident, bias=cvec_sb,
                                 scale=1.0)

            # ---- transpose to batch-major: column n=bl*128+p is batch 4p+bl -
            oTp = ps_ot.tile([128, 4, TO], f32)
            for blk in range(4):
                nc.tensor.transpose(
                    oTp[:, blk, :], osb[:, blk * 128:(blk + 1) * 128], id22_sb)
            outT = otpool.tile([128, 4, TO], f32)
            nc.scalar.copy(outT, oTp)
            # outT[p, blk, :] is batch 4p + blk -> rows (p blk) in order
            dst = out_ext[oct_i * BOCT:(oct_i + 1) * BOCT, :].rearrange(
                "(p blk) to -> p (blk to)", p=128)
            nc.sync.dma_start(out=dst, in_=outT.rearrange("p blk to -> p (blk to)"))

    nc.finalize()
    return nc


def _get_nc():
    if "nc" not in _CACHE:
        _CACHE["nc"] = _build_nc()
    return _CACHE["nc"]


def _fold_weights(W1, b1, Wv, pos_enc, Wo, bo):
    Wvo = Wv.astype(np.float64) @ Wo.astype(np.float64)          # (32, 2)
    t_idx = np.arange(S)
    M = (np.abs(t_idx[:, None] - t_idx[None, :]) <= A).astype(np.float64)  # (t, s)
    # W2[(s-s0)*32+h, g*22 + t*2+o] = M[t, s] * Wvo[h, o]
    w2 = np.zeros((128, 3 * TO), np.float64)
    for g, (s0, s1) in enumerate(KCH):
        blk = np.einsum("st,ho->shto", M.T[s0:s1], Wvo).reshape(
            (s1 - s0) * H, TO)
        w2[:(s1 - s0) * H, g * TO:(g + 1) * TO] = blk
    pos = pos_enc.reshape(S, H).astype(np.float64)
    cvec = (M @ pos) @ Wvo + bo.reshape(1, O).astype(np.float64)  # (t, o)
    b1r = np.tile(b1.reshape(1, H), (4, 1)).reshape(128, 1)
    w1hi = W1.astype(np.float16)
    w1lo = (W1.astype(np.float64) - w1hi.astype(np.float64)).astype(np.float16)
    return (w2.astype(np.float32), cvec.reshape(TO, 1).astype(np.float32),
            b1r.astype(np.float32), w1hi, w1lo)


def kernel(x, W1, b1, Wq, Wk, Wv, pos_enc, Wo, bo):
    from concourse.bass_utils import run_bass_kernel_spmd

    x = np.asarray(x, dtype=np.float32)
    assert x.shape == (B, S, D), x.shape
    w2, cvec, b1r, w1hi, w1lo = _fold_weights(
        np.asarray(W1, np.float32), np.asarray(b1, np.float32),
        np.asarray(Wv, np.float32), np.asarray(pos_enc, np.float32),
        np.asarray(Wo, np.float32), np.asarray(bo, np.float32))
    id22 = np.eye(TO, dtype=np.float32)

    xf = x.reshape(B * S, D)
    xhi = xf.astype(np.float16)
    xlo = (xf - xhi.astype(np.float32)).astype(np.float16)

    nc = _get_nc()
    in_maps = []
    for i in range(NCORES):
        sl = slice(i * NTOK, (i + 1) * NTOK)
        in_maps.append({
            "xhi": np.ascontiguousarray(xhi[sl]),
            "xlo": np.ascontiguousarray(xlo[sl]),
            "w1hi": w1hi, "w1lo": w1lo, "w2": w2, "b1r": b1r,
            "cvec": cvec, "id22": id22,
        })

    res = run_bass_kernel_spmd(nc, in_maps, list(range(NCORES)), trace=False)
    _CACHE["last_result"] = res

    out = np.concatenate([r["out"] for r in res.results], axis=0)
    out = out.reshape(B, S, O)
    attn_w = np.ones((B, S, 1, WIN), dtype=np.float32)
    return out, attn_w


# revision 14
# speedup vs baseline: 1.6069x; 1.0111x over previous
"""Trainium2 Bass kernel for nn_ExRestSelfAtten (sparse window attention).

Math reduction (exact):
  reference softmax is over a singleton axis -> attn_w == ones exactly,
  so Wq/Wk are dead and
    out[b,t,o] = sum_s sum_h M[t,s] * relu(x@W1+b1)[b,s,h] * Wvo[h,o] + c[t,o]
  with M[t,s] = 1{|t-s|<=5}, Wvo = Wv@Wo, c folding pos_enc/bo.

Device pipeline per core (4096 batches = 45056 tokens, 8 octants of 512):
  - x is split on host into fp16 hi/lo planes (x = hi + lo + O(2^-22));
    both planes are DMA-xbar-transposed into SBUF feature-major.
  - MM1 (fp16, 3 terms: hi*W1hi + hi*W1lo + lo*W1hi) contracts f=128 per
    sequence position s, writing PSUM partition strips so partitions become
    (s%4)*32+h; relu+b1 -> R2[g] (g = s//4).
  - MM2 (f32) contracts (s,h) in 3 K-chunks with W2g[(s,h),(t,o)] =
    M[t,s]*Wvo[h,o], accumulating out[(t,o), batch] in PSUM.
  - +c bias, PE-transpose to batch-major, DMA out.
"""

import os
import sys
import numpy as np
from contextlib import ExitStack

sys.path.insert(0, "/opt/trn_rl_repo")

B, S, D, H, O = 32768, 11, 128, 32, 2
A = 5
WIN = 2 * A + 1
NCORES = 8
NB = B // NCORES              # 4096 batches per core
NTOK = NB * S                 # 45056 tokens per core
TO = S * O                    # 22 = flattened (t, o)

OCTS = 8                      # pipeline stages per core
BOCT = NB // OCTS             # 512 batches per octant
TOKOCT = BOCT * S             # 5632 tokens per octant
KCH = [(0, 4), (4, 8), (8, 11)]   # s-ranges of the 3 (s,h) K-chunks

_CACHE = {}


def _build_nc():
    import concourse.bass as bass
    import concourse.tile as tile
    from concourse import bacc, mybir

    f32 = mybir.dt.float32
    f16 = mybir.dt.float16
    Relu = mybir.ActivationFunctionType.Relu
    Ident = mybir.ActivationFunctionType.Identity

    nc = bacc.Bacc()
    xhi_ext = nc.dram_tensor("xhi", [NTOK, D], f16, kind="ExternalInput")
    xlo_ext = nc.dram_tensor("xlo", [NTOK, D], f16, kind="ExternalInput")
    w1hi_ext = nc.dram_tensor("w1hi", [D, H], f16, kind="ExternalInput")
    w1lo_ext = nc.dram_tensor("w1lo", [D, H], f16, kind="ExternalInput")
    w2_ext = nc.dram_tensor("w2", [128, 3 * TO], f32, kind="ExternalInput")
    b1_ext = nc.dram_tensor("b1r", [128, 1], f32, kind="ExternalInput")
    cvec_ext = nc.dram_tensor("cvec", [TO, 1], f32, kind="ExternalInput")
    id22_ext = nc.dram_tensor("id22", [TO, TO], f32, kind="ExternalInput")
    out_ext = nc.dram_tensor("out", [NB, TO], f32, kind="ExternalOutput")

    with tile.TileContext(nc) as tc, ExitStack() as ctx:
        consts = ctx.enter_context(tc.tile_pool(name="consts", bufs=1))
        xtpool = ctx.enter_context(tc.tile_pool(name="xtpool", bufs=3))
        r2pool = ctx.enter_context(tc.tile_pool(name="r2pool", bufs=3))
        osbpool = ctx.enter_context(tc.tile_pool(name="osbpool", bufs=2))
        otpool = ctx.enter_context(tc.tile_pool(name="otpool", bufs=2))
        ps_r2 = ctx.enter_context(tc.tile_pool(name="ps_r2", bufs=2, space="PSUM"))
        ps_o2 = ctx.enter_context(tc.tile_pool(name="ps_o2", bufs=1, space="PSUM"))
        ps_ot = ctx.enter_context(tc.tile_pool(name="ps_ot", bufs=1, space="PSUM"))

        w1hi_sb = consts.tile([D, H], f16)
        nc.sync.dma_start(out=w1hi_sb, in_=w1hi_ext[:])
        w1lo_sb = consts.tile([D, H], f16)
        nc.sync.dma_start(out=w1lo_sb, in_=w1lo_ext[:])
        w2_sb = consts.tile([128, 3 * TO], f32)
        nc.sync.dma_start(out=w2_sb, in_=w2_ext[:])
        b1_sb = consts.tile([128, 1], f32)
        nc.sync.dma_start(out=b1_sb, in_=b1_ext[:])
        cvec_sb = consts.tile([TO, 1], f32)
        nc.sync.dma_start(out=cvec_sb, in_=cvec_ext[:])
        id22_sb = consts.tile([TO, TO], f32)
        nc.sync.dma_start(out=id22_sb, in_=id22_ext[:])

        for oct_i in range(OCTS):
            tok0 = oct_i * TOKOCT
            # ---- xbar-transposed loads: (5632 tok, 128 f) -> (128 f, 5632) ----
            xth = xtpool.tile([D, TOKOCT], f16)
            nc.sync.dma_start_transpose(
                out=xth, in_=xhi_ext[tok0:tok0 + TOKOCT, :])
            xtl = xtpool.tile([D, TOKOCT], f16)
            nc.sync.dma_start_transpose(
                out=xtl, in_=xlo_ext[tok0:tok0 + TOKOCT, :])

            # token t = 44p + 11bl + s ; output column n = bl*128 + p = batch
            # (4p+bl).  View: (f, p, bl, s)
            xthv = xth.rearrange("f (p bl s) -> f s bl p", p=128, bl=4, s=S)
            xtlv = xtl.rearrange("f (p bl s) -> f s bl p", p=128, bl=4, s=S)

            # ---- MM1 into (s%4)*32+h partition strips, one PSUM tile per g --
            r2ps = [ps_r2.tile([128, BOCT], f32, name=f"r2ps{g}_{oct_i}",
                               tag=f"r2ps{g}")
                    for g in range(3)]
            # group by stationary operand to minimize weight reloads
            for phase, (wsb, xv, st, sp) in enumerate([
                    (w1hi_sb, xthv, True, False),
                    (w1hi_sb, xtlv, False, False),
                    (w1lo_sb, xthv, False, True)]):
                for s in range(S):
                    g, sm = s // 4, s % 4
                    nc.tensor.matmul(
                        r2ps[g][32 * sm:32 * sm + 32, :], wsb, xv[:, s],
                        start=st, stop=sp, tile_position=(0, 32 * sm))

            # ---- relu + b1 -> SBUF ----
            r2 = [r2pool.tile([128, BOCT], f32, name=f"r2{g}_{oct_i}",
                              tag=f"r2{g}")
                  for g in range(3)]
            for g, (s0, s1) in enumerate(KCH):
                np_ = 32 * (s1 - s0)
                nc.scalar.activation(
                    out=r2[g][:np_, :], in_=r2ps[g][:np_, :], func=Relu,
                    bias=b1_sb[:np_], scale=1.0)

            # ---- MM2: 3 K-chunks over (s,h) ----
            o2 = ps_o2.tile([TO, BOCT], f32)
            for g, (s0, s1) in enumerate(KCH):
                np_ = 32 * (s1 - s0)
                nc.tensor.matmul(
                    o2, w2_sb[:np_, g * TO:(g + 1) * TO], r2[g][:np_, :],
                    start=(g == 0), stop=(g == 2))
            osb = osbpool.tile([TO, BOCT], f32)
            nc.scalar.activation(out=osb, in_=o2, func=Ident, bias=cvec_sb,
                                 scale=1.0)

            # ---- transpose to batch-major: column n=bl*128+p is batch 4p+bl -
            oTp = ps_ot.tile([128, 4, TO], f32)
            for blk in range(4):
                nc.tensor.transpose(
                    oTp[:, blk, :], osb[:, blk * 128:(blk + 1) * 128], id22_sb)
            outT = otpool.tile([128, 4, TO], f32)
            nc.scalar.copy(outT, oTp)
            # outT[p, blk, :] is batch 4p + blk -> rows (p blk) in order
            dst = out_ext[oct_i * BOCT:(oct_i + 1) * BOCT, :].rearrange(
                "(p blk) to -> p (blk to)", p=128)
            nc.sync.dma_start(out=dst, in_=outT.rearrange("p blk to -> p (blk to)"))

    nc.finalize()
    return nc


def _get_nc():
    if "nc" not in _CACHE:
        _CACHE["nc"] = _build_nc()
    return _CACHE["nc"]


def _fold_weights(W1, b1, Wv, pos_enc, Wo, bo):
    Wvo = Wv.astype(np.float64) @ Wo.astype(np.float64)          # (32, 2)
    t_idx = np.arange(S)
    M = (np.abs(t_idx[:, None] - t_idx[None, :]) <= A).astype(np.float64)  # (t, s)
    # W2[(s-s0)*32+h, g*22 + t*2+o] = M[t, s] * Wvo[h, o]
    w2 = np.zeros((128, 3 * TO), np.float64)
    for g, (s0, s1) in enumerate(KCH):
        blk = np.einsum("st,ho->shto", M.T[s0:s1], Wvo).reshape(
            (s1 - s0) * H, TO)
        w2[:(s1 - s0) * H, g * TO:(g + 1) * TO] = blk
    pos = pos_enc.reshape(S, H).astype(np.float64)
    cvec = (M @ pos) @ Wvo + bo.reshape(1, O).astype(np.float64)  # (t, o)
    b1r = np.tile(b1.reshape(1, H), (4, 1)).reshape(128, 1)
    w1hi = W1.astype(np.float16)
    w1lo = (W1.astype(np.float64) - w1hi.astype(np.float64)).astype(np.float16)
    return (w2.astype(np.float32), cvec.reshape(TO, 1).astype(np.float32),
            b1r.astype(np.float32), w1hi, w1lo)


def kernel(x, W1, b1, Wq, Wk, Wv, pos_enc, Wo, bo):
    from concourse.bass_utils import run_bass_kernel_spmd

    x = np.asarray(x, dtype=np.float32)
    assert x.shape == (B, S, D), x.shape
    w2, cvec, b1r, w1hi, w1lo = _fold_weights(
        np.asarray(W1, np.float32), np.asarray(b1, np.float32),
        np.asarray(Wv, np.float32), np.asarray(pos_enc, np.float32),
        np.asarray(Wo, np.float32), np.asarray(bo, np.float32))
    id22 = np.eye(TO, dtype=np.float32)

    xf = x.reshape(B * S, D)
    xhi = xf.astype(np.float16)
    xlo = (xf - xhi.astype(np.float32)).astype(np.float16)

    nc = _get_nc()
    in_maps = []
    for i in range(NCORES):
        sl = slice(i * NTOK, (i + 1) * NTOK)
        in_maps.append({
            "xhi": np.ascontiguousarray(xhi[sl]),
            "xlo": np.ascontiguousarray(xlo[sl]),
            "w1hi": w1hi, "w1lo": w1lo, "w2": w2, "b1r": b1r,
            "cvec": cvec, "id22": id22,
        })

    res = run_bass_kernel_spmd(nc, in_maps, list(range(NCORES)), trace=False)
    _CACHE["last_result"] = res

    out = np.concatenate([r["out"] for r in res.results], axis=0)
    out = out.reshape(B, S, O)
    attn_w = np.ones((B, S, 1, WIN), dtype=np.float32)
    return out, attn_w


# revision 16
# speedup vs baseline: 1.6157x; 1.0055x over previous
"""Trainium2 Bass kernel for nn_ExRestSelfAtten (sparse window attention).

Math reduction (exact):
  reference softmax is over a singleton axis -> attn_w == ones exactly,
  so Wq/Wk are dead and
    out[b,t,o] = sum_s sum_h M[t,s] * relu(x@W1+b1)[b,s,h] * Wvo[h,o] + c[t,o]
  with M[t,s] = 1{|t-s|<=5}, Wvo = Wv@Wo, c folding pos_enc/bo.

Device pipeline per core (4096 batches = 45056 tokens, 8 octants of 512):
  - x is split on host into fp16 hi/lo planes (x = hi + lo + O(2^-22));
    both planes are DMA-xbar-transposed into SBUF feature-major.
  - MM1 (fp16, 3 terms: hi*W1hi + hi*W1lo + lo*W1hi) contracts f=128 per
    sequence position s, writing PSUM partition strips so partitions become
    (s%4)*32+h; relu+b1 -> R2[g] (g = s//4).
  - MM2 (f32) contracts (s,h) in 3 K-chunks with W2g[(s,h),(t,o)] =
    M[t,s]*Wvo[h,o], accumulating out[(t,o), batch] in PSUM.
  - +c bias, PE-transpose to batch-major, DMA out.
"""

import os
import sys
import numpy as np
from contextlib import ExitStack

sys.path.insert(0, "/opt/trn_rl_repo")

B, S, D, H, O = 32768, 11, 128, 32, 2
A = 5
WIN = 2 * A + 1
NCORES = 8
NB = B // NCORES              # 4096 batches per core
NTOK = NB * S                 # 45056 tokens per core
TO = S * O                    # 22 = flattened (t, o)

OCTS = 8                      # pipeline stages per core
BOCT = NB // OCTS             # 512 batches per octant
TOKOCT = BOCT * S             # 5632 tokens per octant
KCH = [(0, 4), (4, 8), (8, 11)]   # s-ranges of the 3 (s,h) K-chunks

_CACHE = {}


def _build_nc():
    import concourse.bass as bass
    import concourse.tile as tile
    from concourse import bacc, mybir

    f32 = mybir.dt.float32
    f16 = mybir.dt.float16
    Relu = mybir.ActivationFunctionType.Relu
    Ident = mybir.ActivationFunctionType.Identity

    nc = bacc.Bacc()
    xhi_ext = nc.dram_tensor("xhi", [NTOK, D], f16, kind="ExternalInput")
    xlo_ext = nc.dram_tensor("xlo", [NTOK, D], f16, kind="ExternalInput")
    w1hi_ext = nc.dram_tensor("w1hi", [D, H], f16, kind="ExternalInput")
    w1lo_ext = nc.dram_tensor("w1lo", [D, H], f16, kind="ExternalInput")
    w2_ext = nc.dram_tensor("w2", [128, 3 * TO], f32, kind="ExternalInput")
    b1_ext = nc.dram_tensor("b1r", [128, 1], f32, kind="ExternalInput")
    cvec_ext = nc.dram_tensor("cvec", [TO, 1], f32, kind="ExternalInput")
    id22_ext = nc.dram_tensor("id22", [TO, TO], f32, kind="ExternalInput")
    out_ext = nc.dram_tensor("out", [NB, TO], f32, kind="ExternalOutput")

    with tile.TileContext(nc) as tc, ExitStack() as ctx:
        consts = ctx.enter_context(tc.tile_pool(name="consts", bufs=1))
        xtpool = ctx.enter_context(tc.tile_pool(name="xtpool", bufs=3))
        r2pool = ctx.enter_context(tc.tile_pool(name="r2pool", bufs=3))
        osbpool = ctx.enter_context(tc.tile_pool(name="osbpool", bufs=2))
        otpool = ctx.enter_context(tc.tile_pool(name="otpool", bufs=2))
        ps_r2 = ctx.enter_context(tc.tile_pool(name="ps_r2", bufs=2, space="PSUM"))
        ps_o2 = ctx.enter_context(tc.tile_pool(name="ps_o2", bufs=1, space="PSUM"))
        ps_ot = ctx.enter_context(tc.tile_pool(name="ps_ot", bufs=1, space="PSUM"))

        w1hi_sb = consts.tile([D, H], f16)
        nc.sync.dma_start(out=w1hi_sb, in_=w1hi_ext[:])
        w1lo_sb = consts.tile([D, H], f16)
        nc.scalar.dma_start(out=w1lo_sb, in_=w1lo_ext[:])
        w2_sb = consts.tile([128, 3 * TO], f32)
        nc.sync.dma_start(out=w2_sb, in_=w2_ext[:])
        b1_sb = consts.tile([128, 1], f32)
        nc.scalar.dma_start(out=b1_sb, in_=b1_ext[:])
        cvec_sb = consts.tile([TO, 1], f32)
        nc.sync.dma_start(out=cvec_sb, in_=cvec_ext[:])
        id22_sb = consts.tile([TO, TO], f32)
        nc.scalar.dma_start(out=id22_sb, in_=id22_ext[:])

        for oct_i in range(OCTS):
            tok0 = oct_i * TOKOCT
            # ---- xbar-transposed loads: (5632 tok, 128 f) -> (128 f, 5632) ----
            xth = xtpool.tile([D, TOKOCT], f16)
            nc.sync.dma_start_transpose(
                out=xth, in_=xhi_ext[tok0:tok0 + TOKOCT, :])
            xtl = xtpool.tile([D, TOKOCT], f16)
            nc.sync.dma_start_transpose(
                out=xtl, in_=xlo_ext[tok0:tok0 + TOKOCT, :])

            # token t = 44p + 11bl + s ; output column n = bl*128 + p = batch
            # (4p+bl).  View: (f, p, bl, s)
            xthv = xth.rearrange("f (p bl s) -> f s bl p", p=128, bl=4, s=S)
            xtlv = xtl.rearrange("f (p bl s) -> f s bl p", p=128, bl=4, s=S)

            # ---- MM1 into (s%4)*32+h partition strips, one PSUM tile per g --
            r2ps = [ps_r2.tile([128, BOCT], f32, name=f"r2ps{g}_{oct_i}",
                               tag=f"r2ps{g}")
                    for g in range(3)]
            # group by stationary operand to minimize weight reloads
            for phase, (wsb, xv, st, sp) in enumerate([
                    (w1hi_sb, xthv, True, False),
                    (w1hi_sb, xtlv, False, False),
                    (w1lo_sb, xthv, False, True)]):
                for s in range(S):
                    g, sm = s // 4, s % 4
                    nc.tensor.matmul(
                        r2ps[g][32 * sm:32 * sm + 32, :], wsb, xv[:, s],
                        start=st, stop=sp, tile_position=(0, 32 * sm))

            # ---- relu + b1 -> SBUF ----
            r2 = [r2pool.tile([128, BOCT], f32, name=f"r2{g}_{oct_i}",
                              tag=f"r2{g}")
                  for g in range(3)]
            for g, (s0, s1) in enumerate(KCH):
                np_ = 32 * (s1 - s0)
                nc.scalar.activation(
                    out=r2[g][:np_, :], in_=r2ps[g][:np_, :], func=Relu,
                    bias=b1_sb[:np_], scale=1.0)

            # ---- MM2: 3 K-chunks over (s,h) ----
            o2 = ps_o2.tile([TO, BOCT], f32)
            for g, (s0, s1) in enumerate(KCH):
                np_ = 32 * (s1 - s0)
                nc.tensor.matmul(
                    o2, w2_sb[:np_, g * TO:(g + 1) * TO], r2[g][:np_, :],
                    start=(g == 0), stop=(g == 2))
            osb = osbpool.tile([TO, BOCT], f32)
            nc.scalar.activation(out=osb, in_=o2, func=Ident, bias=cvec_sb,
                                 scale=1.0)

            # ---- transpose to batch-major: column n=bl*128+p is batch 4p+bl -
            oTp = ps_ot.tile([128, 4, TO], f32)
            for blk in range(4):
                nc.tensor.transpose(
                    oTp[:, blk, :], osb[:, blk * 128:(blk + 1) * 128], id22_sb)
            outT = otpool.tile([128, 4, TO], f32)
            nc.scalar.copy(outT, oTp)
            # outT[p, blk, :] is batch 4p + blk -> rows (p blk) in order
            dst = out_ext[oct_i * BOCT:(oct_i + 1) * BOCT, :].rearrange(
                "(p blk) to -> p (blk to)", p=128)
            nc.sync.dma_start(out=dst, in_=outT.rearrange("p blk to -> p (blk to)"))

    nc.finalize()
    return nc


def _get_nc():
    if "nc" not in _CACHE:
        _CACHE["nc"] = _build_nc()
    return _CACHE["nc"]


def _fold_weights(W1, b1, Wv, pos_enc, Wo, bo):
    Wvo = Wv.astype(np.float64) @ Wo.astype(np.float64)          # (32, 2)
    t_idx = np.arange(S)
    M = (np.abs(t_idx[:, None] - t_idx[None, :]) <= A).astype(np.float64)  # (t, s)
    # W2[(s-s0)*32+h, g*22 + t*2+o] = M[t, s] * Wvo[h, o]
    w2 = np.zeros((128, 3 * TO), np.float64)
    for g, (s0, s1) in enumerate(KCH):
        blk = np.einsum("st,ho->shto", M.T[s0:s1], Wvo).reshape(
            (s1 - s0) * H, TO)
        w2[:(s1 - s0) * H, g * TO:(g + 1) * TO] = blk
    pos = pos_enc.reshape(S, H).astype(np.float64)
    cvec = (M @ pos) @ Wvo + bo.reshape(1, O).astype(np.float64)  # (t, o)
    b1r = np.tile(b1.reshape(1, H), (4, 1)).reshape(128, 1)
    w1hi = W1.astype(np.float16)
    w1lo = (W1.astype(np.float64) - w1hi.astype(np.float64)).astype(np.float16)
    return (w2.astype(np.float32), cvec.reshape(TO, 1).astype(np.float32),
            b1r.astype(np.float32), w1hi, w1lo)


def kernel(x, W1, b1, Wq, Wk, Wv, pos_enc, Wo, bo):
    from concourse.bass_utils import run_bass_kernel_spmd

    x = np.asarray(x, dtype=np.float32)
    assert x.shape == (B, S, D), x.shape
    w2, cvec, b1r, w1hi, w1lo = _fold_weights(
        np.asarray(W1, np.float32), np.asarray(b1, np.float32),
        np.asarray(Wv, np.float32), np.asarray(pos_enc, np.float32),
        np.asarray(Wo, np.float32), np.asarray(bo, np.float32))
    id22 = np.eye(TO, dtype=np.float32)

    xf = x.reshape(B * S, D)
    xhi = xf.astype(np.float16)
    xlo = (xf - xhi.astype(np.float32)).astype(np.float16)

    nc = _get_nc()
    in_maps = []
    for i in range(NCORES):
        sl = slice(i * NTOK, (i + 1) * NTOK)
        in_maps.append({
            "xhi": np.ascontiguousarray(xhi[sl]),
            "xlo": np.ascontiguousarray(xlo[sl]),
            "w1hi": w1hi, "w1lo": w1lo, "w2": w2, "b1r": b1r,
            "cvec": cvec, "id22": id22,
        })

    res = run_bass_kernel_spmd(nc, in_maps, list(range(NCORES)), trace=False)
    _CACHE["last_result"] = res

    out = np.concatenate([r["out"] for r in res.results], axis=0)
    out = out.reshape(B, S, O)
    attn_w = np.ones((B, S, 1, WIN), dtype=np.float32)
    return out, attn_w


# revision 17
# speedup vs baseline: 1.6298x; 1.0087x over previous
"""Trainium2 Bass kernel for nn_ExRestSelfAtten (sparse window attention).

Math reduction (exact):
  reference softmax is over a singleton axis -> attn_w == ones exactly,
  so Wq/Wk are dead and
    out[b,t,o] = sum_s sum_h M[t,s] * relu(x@W1+b1)[b,s,h] * Wvo[h,o] + c[t,o]
  with M[t,s] = 1{|t-s|<=5}, Wvo = Wv@Wo, c folding pos_enc/bo.

Device pipeline per core (4096 batches = 45056 tokens, 8 octants of 512):
  - x is split on host into fp16 hi/lo planes (x = hi + lo + O(2^-22));
    both planes are DMA-xbar-transposed into SBUF feature-major.
  - MM1 (fp16, 3 terms: hi*W1hi + hi*W1lo + lo*W1hi) contracts f=128 per
    sequence position s, writing PSUM partition strips so partitions become
    (s%4)*32+h; relu+b1 -> R2[g] (g = s//4).
  - MM2 (f32) contracts (s,h) in 3 K-chunks with W2g[(s,h),(t,o)] =
    M[t,s]*Wvo[h,o], accumulating out[(t,o), batch] in PSUM.
  - +c bias, PE-transpose to batch-major, DMA out.
"""

import os
import sys
import numpy as np
from contextlib import ExitStack

sys.path.insert(0, "/opt/trn_rl_repo")

B, S, D, H, O = 32768, 11, 128, 32, 2
A = 5
WIN = 2 * A + 1
NCORES = 8
NB = B // NCORES              # 4096 batches per core
NTOK = NB * S                 # 45056 tokens per core
TO = S * O                    # 22 = flattened (t, o)

OCTS = 8                      # pipeline stages per core
BOCT = NB // OCTS             # 512 batches per octant
TOKOCT = BOCT * S             # 5632 tokens per octant
KCH = [(0, 4), (4, 8), (8, 11)]   # s-ranges of the 3 (s,h) K-chunks

_CACHE = {}


def _build_nc():
    import concourse.bass as bass
    import concourse.tile as tile
    from concourse import bacc, mybir

    f32 = mybir.dt.float32
    f16 = mybir.dt.float16
    Relu = mybir.ActivationFunctionType.Relu
    Ident = mybir.ActivationFunctionType.Identity

    nc = bacc.Bacc()
    xcat_ext = nc.dram_tensor("xcat", [2 * NTOK, D], f16, kind="ExternalInput")
    w1hi_ext = nc.dram_tensor("w1hi", [D, H], f16, kind="ExternalInput")
    w1lo_ext = nc.dram_tensor("w1lo", [D, H], f16, kind="ExternalInput")
    w2_ext = nc.dram_tensor("w2", [128, 3 * TO], f32, kind="ExternalInput")
    b1_ext = nc.dram_tensor("b1r", [128, 1], f32, kind="ExternalInput")
    cvec_ext = nc.dram_tensor("cvec", [TO, 1], f32, kind="ExternalInput")
    id22_ext = nc.dram_tensor("id22", [TO, TO], f32, kind="ExternalInput")
    out_ext = nc.dram_tensor("out", [NB, TO], f32, kind="ExternalOutput")

    with tile.TileContext(nc) as tc, ExitStack() as ctx:
        consts = ctx.enter_context(tc.tile_pool(name="consts", bufs=1))
        xtpool = ctx.enter_context(tc.tile_pool(name="xtpool", bufs=3))
        r2pool = ctx.enter_context(tc.tile_pool(name="r2pool", bufs=3))
        osbpool = ctx.enter_context(tc.tile_pool(name="osbpool", bufs=2))
        otpool = ctx.enter_context(tc.tile_pool(name="otpool", bufs=2))
        ps_r2 = ctx.enter_context(tc.tile_pool(name="ps_r2", bufs=2, space="PSUM"))
        ps_o2 = ctx.enter_context(tc.tile_pool(name="ps_o2", bufs=1, space="PSUM"))
        ps_ot = ctx.enter_context(tc.tile_pool(name="ps_ot", bufs=1, space="PSUM"))

        w1hi_sb = consts.tile([D, H], f16)
        nc.sync.dma_start(out=w1hi_sb, in_=w1hi_ext[:])
        w1lo_sb = consts.tile([D, H], f16)
        nc.scalar.dma_start(out=w1lo_sb, in_=w1lo_ext[:])
        w2_sb = consts.tile([128, 3 * TO], f32)
        nc.sync.dma_start(out=w2_sb, in_=w2_ext[:])
        b1_sb = consts.tile([128, 1], f32)
        nc.scalar.dma_start(out=b1_sb, in_=b1_ext[:])
        cvec_sb = consts.tile([TO, 1], f32)
        nc.sync.dma_start(out=cvec_sb, in_=cvec_ext[:])
        id22_sb = consts.tile([TO, TO], f32)
        nc.scalar.dma_start(out=id22_sb, in_=id22_ext[:])

        for oct_i in range(OCTS):
            tok0 = oct_i * 2 * TOKOCT
            # ---- one xbar-transposed load: rows [hi(5632); lo(5632)] ----
            xt2 = xtpool.tile([D, 2 * TOKOCT], f16)
            nc.sync.dma_start_transpose(
                out=xt2, in_=xcat_ext[tok0:tok0 + 2 * TOKOCT, :])
            xth = xt2[:, :TOKOCT]
            xtl = xt2[:, TOKOCT:]

            # token t = 44p + 11bl + s ; output column n = bl*128 + p = batch
            # (4p+bl).  View: (f, p, bl, s)
            xthv = xth.rearrange("f (p bl s) -> f s bl p", p=128, bl=4, s=S)
            xtlv = xtl.rearrange("f (p bl s) -> f s bl p", p=128, bl=4, s=S)

            # ---- MM1 into (s%4)*32+h partition strips, one PSUM tile per g --
            r2ps = [ps_r2.tile([128, BOCT], f32, name=f"r2ps{g}_{oct_i}",
                               tag=f"r2ps{g}")
                    for g in range(3)]
            # group by stationary operand to minimize weight reloads
            for phase, (wsb, xv, st, sp) in enumerate([
                    (w1hi_sb, xthv, True, False),
                    (w1hi_sb, xtlv, False, False),
                    (w1lo_sb, xthv, False, True)]):
                for s in range(S):
                    g, sm = s // 4, s % 4
                    nc.tensor.matmul(
                        r2ps[g][32 * sm:32 * sm + 32, :], wsb, xv[:, s],
                        start=st, stop=sp, tile_position=(0, 32 * sm))

            # ---- relu + b1 -> SBUF ----
            r2 = [r2pool.tile([128, BOCT], f32, name=f"r2{g}_{oct_i}",
                              tag=f"r2{g}")
                  for g in range(3)]
            for g, (s0, s1) in enumerate(KCH):
                np_ = 32 * (s1 - s0)
                nc.scalar.activation(
                    out=r2[g][:np_, :], in_=r2ps[g][:np_, :], func=Relu,
                    bias=b1_sb[:np_], scale=1.0)

            # ---- MM2: 3 K-chunks over (s,h) ----
            o2 = ps_o2.tile([TO, BOCT], f32)
            for g, (s0, s1) in enumerate(KCH):
                np_ = 32 * (s1 - s0)
                nc.tensor.matmul(
                    o2, w2_sb[:np_, g * TO:(g + 1) * TO], r2[g][:np_, :],
                    start=(g == 0), stop=(g == 2))
            osb = osbpool.tile([TO, BOCT], f32)
            nc.scalar.activation(out=osb, in_=o2, func=Ident, bias=cvec_sb,
                                 scale=1.0)

            # ---- transpose to batch-major: column n=bl*128+p is batch 4p+bl -
            oTp = ps_ot.tile([128, 4, TO], f32)
            for blk in range(4):
                nc.tensor.transpose(
                    oTp[:, blk, :], osb[:, blk * 128:(blk + 1) * 128], id22_sb)
            outT = otpool.tile([128, 4, TO], f32)
            nc.scalar.copy(outT, oTp)
            # outT[p, blk, :] is batch 4p + blk -> rows (p blk) in order
            dst = out_ext[oct_i * BOCT:(oct_i + 1) * BOCT, :].rearrange(
                "(p blk) to -> p (blk to)", p=128)
            nc.scalar.dma_start(out=dst, in_=outT.rearrange("p blk to -> p (blk to)"))

    nc.finalize()
    return nc


def _get_nc():
    if "nc" not in _CACHE:
        _CACHE["nc"] = _build_nc()
    return _CACHE["nc"]


def _fold_weights(W1, b1, Wv, pos_enc, Wo, bo):
    Wvo = Wv.astype(np.float64) @ Wo.astype(np.float64)          # (32, 2)
    t_idx = np.arange(S)
    M = (np.abs(t_idx[:, None] - t_idx[None, :]) <= A).astype(np.float64)  # (t, s)
    # W2[(s-s0)*32+h, g*22 + t*2+o] = M[t, s] * Wvo[h, o]
    w2 = np.zeros((128, 3 * TO), np.float64)
    for g, (s0, s1) in enumerate(KCH):
        blk = np.einsum("st,ho->shto", M.T[s0:s1], Wvo).reshape(
            (s1 - s0) * H, TO)
        w2[:(s1 - s0) * H, g * TO:(g + 1) * TO] = blk
    pos = pos_enc.reshape(S, H).astype(np.float64)
    cvec = (M @ pos) @ Wvo + bo.reshape(1, O).astype(np.float64)  # (t, o)
    b1r = np.tile(b1.reshape(1, H), (4, 1)).reshape(128, 1)
    w1hi = W1.astype(np.float16)
    w1lo = (W1.astype(np.float64) - w1hi.astype(np.float64)).astype(np.float16)
    return (w2.astype(np.float32), cvec.reshape(TO, 1).astype(np.float32),
            b1r.astype(np.float32), w1hi, w1lo)


def kernel(x, W1, b1, Wq, Wk, Wv, pos_enc, Wo, bo):
    from concourse.bass_utils import run_bass_kernel_spmd

    x = np.asarray(x, dtype=np.float32)
    assert x.shape == (B, S, D), x.shape
    w2, cvec, b1r, w1hi, w1lo = _fold_weights(
        np.asarray(W1, np.float32), np.asarray(b1, np.float32),
        np.asarray(Wv, np.float32), np.asarray(pos_enc, np.float32),
        np.asarray(Wo, np.float32), np.asarray(bo, np.float32))
    id22 = np.eye(TO, dtype=np.float32)

    xf = x.reshape(B * S, D)
    xhi = xf.astype(np.float16)
    xlo = (xf - xhi.astype(np.float32)).astype(np.float16)
    # interleave per octant: [hi_oct; lo_oct] blocks of 5632 rows
    xhi_o = xhi.reshape(NCORES, OCTS, TOKOCT, D)
    xlo_o = xlo.reshape(NCORES, OCTS, TOKOCT, D)
    xcat = np.stack([xhi_o, xlo_o], axis=2).reshape(NCORES, 2 * NTOK, D)

    nc = _get_nc()
    in_maps = []
    for i in range(NCORES):
        in_maps.append({
            "xcat": np.ascontiguousarray(xcat[i]),
            "w1hi": w1hi, "w1lo": w1lo, "w2": w2, "b1r": b1r,
            "cvec": cvec, "id22": id22,
        })

    res = run_bass_kernel_spmd(nc, in_maps, list(range(NCORES)), trace=False)
    _CACHE["last_result"] = res

    out = np.concatenate([r["out"] for r in res.results], axis=0)
    out = out.reshape(B, S, O)
    attn_w = np.ones((B, S, 1, WIN), dtype=np.float32)
    return out, attn_w


# revision 18
# speedup vs baseline: 1.9151x; 1.1751x over previous
"""Trainium2 Bass kernel for nn_ExRestSelfAtten (sparse window attention).

Math reduction (exact):
  reference softmax is over a singleton axis -> attn_w == ones exactly,
  so Wq/Wk are dead and
    out[b,t,o] = sum_s sum_h M[t,s] * relu(x@W1+b1)[b,s,h] * Wvo[h,o] + c[t,o]
  with M[t,s] = 1{|t-s|<=5}, Wvo = Wv@Wo, c folding pos_enc/bo.

Device pipeline per core (4096 batches = 45056 tokens, 8 octants of 512):
  - x is split on host into fp16 hi/lo planes (x = hi + lo + O(2^-22));
    both planes are DMA-xbar-transposed into SBUF feature-major.
  - MM1 (fp16, 3 terms: hi*W1hi + hi*W1lo + lo*W1hi) contracts f=128 per
    sequence position s, writing PSUM partition strips so partitions become
    (s%4)*32+h; relu+b1 -> R2[g] (g = s//4).
  - MM2 (f32) contracts (s,h) in 3 K-chunks with W2g[(s,h),(t,o)] =
    M[t,s]*Wvo[h,o], accumulating out[(t,o), batch] in PSUM.
  - +c bias, PE-transpose to batch-major, DMA out.
"""

import os
import sys
import numpy as np
from contextlib import ExitStack

sys.path.insert(0, "/opt/trn_rl_repo")

B, S, D, H, O = 32768, 11, 128, 32, 2
A = 5
WIN = 2 * A + 1
NCORES = 8
NB = B // NCORES              # 4096 batches per core
NTOK = NB * S                 # 45056 tokens per core
TO = S * O                    # 22 = flattened (t, o)

OCTS = 8                      # pipeline stages per core
BOCT = NB // OCTS             # 512 batches per octant
TOKOCT = BOCT * S             # 5632 tokens per octant
KCH = [(0, 4), (4, 8), (8, 11)]   # s-ranges of the 3 (s,h) K-chunks

_CACHE = {}


def _build_nc():
    import concourse.bass as bass
    import concourse.tile as tile
    from concourse import bacc, mybir

    f32 = mybir.dt.float32
    f16 = mybir.dt.float16
    Relu = mybir.ActivationFunctionType.Relu
    Ident = mybir.ActivationFunctionType.Identity

    nc = bacc.Bacc()
    xcat_ext = nc.dram_tensor("xcat", [2 * NTOK, D], f16, kind="ExternalInput")
    w1hi_ext = nc.dram_tensor("w1hi", [D, H], f16, kind="ExternalInput")
    w1lo_ext = nc.dram_tensor("w1lo", [D, H], f16, kind="ExternalInput")
    w2_ext = nc.dram_tensor("w2", [128, 3 * TO], f32, kind="ExternalInput")
    b1_ext = nc.dram_tensor("b1r", [128, 1], f32, kind="ExternalInput")
    cvec_ext = nc.dram_tensor("cvec", [TO, 1], f32, kind="ExternalInput")
    id22_ext = nc.dram_tensor("id22", [TO, TO], f32, kind="ExternalInput")
    out_ext = nc.dram_tensor("out", [NB, TO], f32, kind="ExternalOutput")

    with tile.TileContext(nc) as tc, ExitStack() as ctx:
        consts = ctx.enter_context(tc.tile_pool(name="consts", bufs=1))
        xtpool = ctx.enter_context(tc.tile_pool(name="xtpool", bufs=3))
        r2pool = ctx.enter_context(tc.tile_pool(name="r2pool", bufs=3))
        osbpool = ctx.enter_context(tc.tile_pool(name="osbpool", bufs=2))
        otpool = ctx.enter_context(tc.tile_pool(name="otpool", bufs=1))
        ps_r2 = ctx.enter_context(tc.tile_pool(name="ps_r2", bufs=2, space="PSUM"))
        ps_o2 = ctx.enter_context(tc.tile_pool(name="ps_o2", bufs=1, space="PSUM"))
        ps_ot = ctx.enter_context(tc.tile_pool(name="ps_ot", bufs=1, space="PSUM"))

        w1hi_sb = consts.tile([D, H], f16)
        nc.sync.dma_start(out=w1hi_sb, in_=w1hi_ext[:])
        w1lo_sb = consts.tile([D, H], f16)
        nc.scalar.dma_start(out=w1lo_sb, in_=w1lo_ext[:])
        w2_sb = consts.tile([128, 3 * TO], f32)
        nc.sync.dma_start(out=w2_sb, in_=w2_ext[:])
        b1_sb = consts.tile([128, 1], f32)
        nc.scalar.dma_start(out=b1_sb, in_=b1_ext[:])
        cvec_sb = consts.tile([TO, 1], f32)
        nc.sync.dma_start(out=cvec_sb, in_=cvec_ext[:])
        id22_sb = consts.tile([TO, TO], f32)
        nc.scalar.dma_start(out=id22_sb, in_=id22_ext[:])

        outbuf = otpool.tile([128, OCTS, 4, TO], f32)

        for oct_i in range(OCTS):
            tok0 = oct_i * 2 * TOKOCT
            # ---- one xbar-transposed load: rows [hi(5632); lo(5632)] ----
            xt2 = xtpool.tile([D, 2 * TOKOCT], f16)
            nc.sync.dma_start_transpose(
                out=xt2, in_=xcat_ext[tok0:tok0 + 2 * TOKOCT, :])
            xth = xt2[:, :TOKOCT]
            xtl = xt2[:, TOKOCT:]

            # token t = 44p + 11bl + s ; output column n = bl*128 + p = batch
            # (4p+bl).  View: (f, p, bl, s)
            xthv = xth.rearrange("f (p bl s) -> f s bl p", p=128, bl=4, s=S)
            xtlv = xtl.rearrange("f (p bl s) -> f s bl p", p=128, bl=4, s=S)

            # ---- MM1 into (s%4)*32+h partition strips, one PSUM tile per g --
            r2ps = [ps_r2.tile([128, BOCT], f32, name=f"r2ps{g}_{oct_i}",
                               tag=f"r2ps{g}")
                    for g in range(3)]
            # group by stationary operand to minimize weight reloads
            for phase, (wsb, xv, st, sp) in enumerate([
                    (w1hi_sb, xthv, True, False),
                    (w1hi_sb, xtlv, False, False),
                    (w1lo_sb, xthv, False, True)]):
                for s in range(S):
                    g, sm = s // 4, s % 4
                    nc.tensor.matmul(
                        r2ps[g][32 * sm:32 * sm + 32, :], wsb, xv[:, s],
                        start=st, stop=sp, tile_position=(0, 32 * sm))

            # ---- relu + b1 -> SBUF ----
            r2 = [r2pool.tile([128, BOCT], f32, name=f"r2{g}_{oct_i}",
                              tag=f"r2{g}")
                  for g in range(3)]
            for g, (s0, s1) in enumerate(KCH):
                np_ = 32 * (s1 - s0)
                nc.scalar.activation(
                    out=r2[g][:np_, :], in_=r2ps[g][:np_, :], func=Relu,
                    bias=b1_sb[:np_], scale=1.0)

            # ---- MM2: 3 K-chunks over (s,h) ----
            o2 = ps_o2.tile([TO, BOCT], f32)
            for g, (s0, s1) in enumerate(KCH):
                np_ = 32 * (s1 - s0)
                nc.tensor.matmul(
                    o2, w2_sb[:np_, g * TO:(g + 1) * TO], r2[g][:np_, :],
                    start=(g == 0), stop=(g == 2))
            osb = osbpool.tile([TO, BOCT], f32)
            nc.scalar.activation(out=osb, in_=o2, func=Ident, bias=cvec_sb,
                                 scale=1.0)

            # ---- transpose to batch-major: column n=bl*128+p is batch 4p+bl -
            oTp = ps_ot.tile([128, 4, TO], f32)
            for blk in range(4):
                nc.tensor.transpose(
                    oTp[:, blk, :], osb[:, blk * 128:(blk + 1) * 128], id22_sb)
            nc.scalar.copy(outbuf[:, oct_i], oTp)

        # one store at the end: row = oct*512 + 4p + blk
        dst = out_ext[:].rearrange(
            "(oct p blk) to -> p oct (blk to)", oct=OCTS, p=128, blk=4)
        nc.sync.dma_start(out=dst, in_=outbuf.rearrange("p o blk to -> p o (blk to)"))

    nc.finalize()
    return nc


def _get_nc():
    if "nc" not in _CACHE:
        _CACHE["nc"] = _build_nc()
    return _CACHE["nc"]


def _fold_weights(W1, b1, Wv, pos_enc, Wo, bo):
    Wvo = Wv.astype(np.float64) @ Wo.astype(np.float64)          # (32, 2)
    t_idx = np.arange(S)
    M = (np.abs(t_idx[:, None] - t_idx[None, :]) <= A).astype(np.float64)  # (t, s)
    # W2[(s-s0)*32+h, g*22 + t*2+o] = M[t, s] * Wvo[h, o]
    w2 = np.zeros((128, 3 * TO), np.float64)
    for g, (s0, s1) in enumerate(KCH):
        blk = np.einsum("st,ho->shto", M.T[s0:s1], Wvo).reshape(
            (s1 - s0) * H, TO)
        w2[:(s1 - s0) * H, g * TO:(g + 1) * TO] = blk
    pos = pos_enc.reshape(S, H).astype(np.float64)
    cvec = (M @ pos) @ Wvo + bo.reshape(1, O).astype(np.float64)  # (t, o)
    b1r = np.tile(b1.reshape(1, H), (4, 1)).reshape(128, 1)
    w1hi = W1.astype(np.float16)
    w1lo = (W1.astype(np.float64) - w1hi.astype(np.float64)).astype(np.float16)
    return (w2.astype(np.float32), cvec.reshape(TO, 1).astype(np.float32),
            b1r.astype(np.float32), w1hi, w1lo)


def kernel(x, W1, b1, Wq, Wk, Wv, pos_enc, Wo, bo):
    from concourse.bass_utils import run_bass_kernel_spmd

    x = np.asarray(x, dtype=np.float32)
    assert x.shape == (B, S, D), x.shape
    w2, cvec, b1r, w1hi, w1lo = _fold_weights(
        np.asarray(W1, np.float32), np.asarray(b1, np.float32),
        np.asarray(Wv, np.float32), np.asarray(pos_enc, np.float32),
        np.asarray(Wo, np.float32), np.asarray(bo, np.float32))
    id22 = np.eye(TO, dtype=np.float32)

    xf = x.reshape(B * S, D)
    xhi = xf.astype(np.float16)
    xlo = (xf - xhi.astype(np.float32)).astype(np.float16)
    # interleave per octant: [hi_oct; lo_oct] blocks of 5632 rows
    xhi_o = xhi.reshape(NCORES, OCTS, TOKOCT, D)
    xlo_o = xlo.reshape(NCORES, OCTS, TOKOCT, D)
    xcat = np.stack([xhi_o, xlo_o], axis=2).reshape(NCORES, 2 * NTOK, D)

    nc = _get_nc()
    in_maps = []
    for i in range(NCORES):
        in_maps.append({
            "xcat": np.ascontiguousarray(xcat[i]),
            "w1hi": w1hi, "w1lo": w1lo, "w2": w2, "b1r": b1r,
            "cvec": cvec, "id22": id22,
        })

    res = run_bass_kernel_spmd(nc, in_maps, list(range(NCORES)), trace=False)
    _CACHE["last_result"] = res

    out = np.concatenate([r["out"] for r in res.results], axis=0)
    out = out.reshape(B, S, O)
    attn_w = np.ones((B, S, 1, WIN), dtype=np.float32)
    return out, attn_w
